# revision 12
# baseline (speedup 1.0000x reference)
"""Distributed causal RoPE attention for Trainium2 (8 NeuronCores).

Problem: nn_CausalRpeAttn — B=2, S=2048, D=1024, H=16, Dh=64, fp32.

Sharding (data + head parallel): core c handles batch c//4 and heads
4*(c%4) .. 4*(c%4)+3 (a 256-wide feature slice). Wq/Wk/Wv are split
column-wise (by output head group), Wo row-wise. Each core writes its
full [1024, 2048] (transposed) bf16 partial output projection (with
bo/4 pre-added); the host unshards by summing the 4 partials per batch
and transposing back. Attention itself is fully independent per
(batch, head), so the only cross-core combination is that final sum.

Key performance structure (v2):
 - Everything bf16 on the wires; fp32 only inside PSUM accumulation and
   the softmax denominator reciprocal.
 - q/k projections feature-major (moving dim 512); v is projected
   POSITION-major directly on the PE (x chunk as the stationary
   operand, Wv as moving) so no transposes are needed; bv is added
   during the PSUM->SBUF evacuation on the DVE against a pre-broadcast
   bias tile.
 - Scores are computed transposed sT[k, q], two heads of a 128-feature
   block run CONCURRENTLY on disjoint PE row groups (lhsT base
   partitions 0 / 64) into different PSUM banks. Causal-trimmed moving
   ranges everywhere.
 - exp on the Scalar engine only (it is the phase-B co-bottleneck);
   all copies/bias-adds live on DVE/GpSimd.
 - v carries an appended ones-row so PV yields the softmax denominator
   row; the pair's PV PSUM tile is evacuated to SBUF fp32 immediately
   (frees banks for the next pair), then reciprocal_approx_fast on the
   denominator row, gpsimd partition-broadcast, and two DVE muls
   produce the normalized bf16 Wo input. Wo runs one q-tile late so the
   PE never waits on that chain.
 - Input DMAs are split small (x in quarters per 128-partition chunk)
   and issued from four different engine queues so the first
   projection matmul starts ~6us in and transfers overlap compute.
"""

import os
import ml_dtypes
import numpy as np

B, S, D, H, DH = 2, 2048, 1024, 16, 64
N_CORES = 8
FPC = 256  # features per core (4 heads)
QT = 512
NQT = S // QT  # 4
NST = S // 512  # 4 s-tiles for projections

_cache = {}
last_run_info = {}


def _build():
    import concourse.bass as bass
    import concourse.mybir as mybir
    import concourse.tile as tile
    from concourse import bacc

    F32 = mybir.dt.float32
    BF16 = mybir.dt.bfloat16
    AOP = mybir.AluOpType
    EXP = mybir.ActivationFunctionType.Exp

    nc = bacc.Bacc("TRN2", target_bir_lowering=False, debug=False,
                   num_devices=N_CORES)

    qkvT_e = nc.dram_tensor("qkvT", [D, S], BF16, kind="ExternalInput").ap()
    wq_e = nc.dram_tensor("wq", [D, FPC], BF16, kind="ExternalInput").ap()
    wk_e = nc.dram_tensor("wk", [D, FPC], BF16, kind="ExternalInput").ap()
    wv_e = nc.dram_tensor("wv", [D, FPC], BF16, kind="ExternalInput").ap()
    wo_e = nc.dram_tensor("wo", [FPC, D], BF16, kind="ExternalInput").ap()
    bq_e = nc.dram_tensor("bq", [FPC], F32, kind="ExternalInput").ap()
    bk_e = nc.dram_tensor("bk", [FPC], F32, kind="ExternalInput").ap()
    bv_e = nc.dram_tensor("bv", [1, FPC], F32, kind="ExternalInput").ap()
    bo_e = nc.dram_tensor("bo", [D], F32, kind="ExternalInput").ap()
    cos2_e = nc.dram_tensor("cos2", [128, S], BF16, kind="ExternalInput").ap()
    sinx_e = nc.dram_tensor("sinx", [128, S], BF16, kind="ExternalInput").ap()
    out_e = nc.dram_tensor("out", [D, S], BF16, kind="ExternalOutput").ap()

    from contextlib import ExitStack
    with tile.TileContext(nc) as tc:
        with ExitStack() as ctx:
            ep = ctx.enter_context
            consts = ep(tc.tile_pool(name="consts", bufs=1))
            xin_pool = ep(tc.tile_pool(name="xin", bufs=1))
            rope_pool = ep(tc.tile_pool(name="rope", bufs=4))
            qb_pool = ep(tc.tile_pool(name="qb", bufs=2))
            qbs_pool = ep(tc.tile_pool(name="qbs", bufs=2))
            tmp_pool = ep(tc.tile_pool(name="tmp", bufs=2))
            vsb_pool = ep(tc.tile_pool(name="vsb", bufs=1))
            probs_pool = ep(tc.tile_pool(name="probs", bufs=4))
            woin_pool = ep(tc.tile_pool(name="woin", bufs=2))
            rec_pool = ep(tc.tile_pool(name="rec", bufs=2))
            rb_pool = ep(tc.tile_pool(name="rb", bufs=2))
            osb_pool = ep(tc.tile_pool(name="osb", bufs=3))
            # PSUM: sc 2x[128,1024] = 4 banks, ps 4x[128,512] = 4 banks.
            sc_pool = ep(tc.tile_pool(name="sc", bufs=2, space="PSUM"))
            ps_pool = ep(tc.tile_pool(name="ps", bufs=4, space="PSUM"))

            # ---- input DMAs, split fine and spread across issue queues ----
            wq_sb = consts.tile([128, 8, FPC], BF16, tag="wq")
            wk_sb = consts.tile([128, 8, FPC], BF16, tag="wk")
            wv_sb = consts.tile([128, 8, FPC], BF16, tag="wv")
            wq_r = wq_e.rearrange("(kt p) f -> p kt f", p=128)
            wk_r = wk_e.rearrange("(kt p) f -> p kt f", p=128)
            wv_r = wv_e.rearrange("(kt p) f -> p kt f", p=128)
            x_all = xin_pool.tile([128, 8, S], BF16, tag="x")

            # sync queue: wq interleaved with the first x quarter so the
            # q-projection can start streaming immediately.
            for kt in range(8):
                nc.sync.dma_start(out=wq_sb[:, kt, :], in_=wq_r[:, kt, :])
                nc.sync.dma_start(out=x_all[:, kt, 0:512],
                                  in_=qkvT_e[kt * 128:(kt + 1) * 128, 0:512])
            for kt in range(8):
                nc.sync.dma_start(out=x_all[:, kt, 512:1024],
                                  in_=qkvT_e[kt * 128:(kt + 1) * 128, 512:1024])

            # scalar queue: x quarter 2, then rope tables
            for kt in range(8):
                nc.sync.dma_start(
                    out=x_all[:, kt, 1024:1536],
                    in_=qkvT_e[kt * 128:(kt + 1) * 128, 1024:1536])

            # scalar queue: rope tables (needed ~15us in)
            cos2_sb = consts.tile([128, S], BF16, tag="cos2")
            sinx_sb = consts.tile([128, S], BF16, tag="sinx")
            for h in (0, 1):
                hs = slice(h * 1024, (h + 1) * 1024)
                nc.sync.dma_start(out=cos2_sb[:, hs], in_=cos2_e[:, hs])
                nc.sync.dma_start(out=sinx_sb[:, hs], in_=sinx_e[:, hs])

            # gpsimd queue: x quarter 3, k/v/o weights + biases
            for kt in range(8):
                nc.sync.dma_start(
                    out=x_all[:, kt, 1536:2048],
                    in_=qkvT_e[kt * 128:(kt + 1) * 128, 1536:2048])
            b_sbs = []
            for name, be in (("bq", bq_e), ("bk", bk_e)):
                t = consts.tile([128, 2], F32, tag=name, name=name)
                nc.sync.dma_start(out=t[:],
                                    in_=be.rearrange("(t p) -> p t", p=128))
                b_sbs.append(t)
            bq_sb, bk_sb = b_sbs
            bv_row = consts.tile([1, FPC], F32, tag="bvr")
            nc.sync.dma_start(out=bv_row[:], in_=bv_e)
            bo_sb = consts.tile([128, 8], F32, tag="bo")
            nc.sync.dma_start(out=bo_sb[:],
                                in_=bo_e.rearrange("(t p) -> p t", p=128))
            for kt in range(0, 8, 2):
                nc.sync.dma_start(out=wk_sb[:, kt:kt + 2, :],
                                    in_=wk_r[:, kt:kt + 2, :])
            for kt in range(0, 8, 2):
                nc.sync.dma_start(out=wv_sb[:, kt:kt + 2, :],
                                    in_=wv_r[:, kt:kt + 2, :])
            wo_sb = consts.tile([128, 2, D], BF16, tag="wo")
            wo_r = wo_e.rearrange("(pt p) f -> p pt f", p=128)
            for pt in range(2):
                nc.sync.dma_start(out=wo_sb[:, pt, :], in_=wo_r[:, pt, :])

            bv_bc = consts.tile([128, FPC], F32, tag="bvb")
            nc.gpsimd.partition_broadcast(bv_bc[:], bv_row[0:1, :])

            # v with ones row: [pos 128, 16 s-tiles, 4 heads, 64+1]
            v_sb = vsb_pool.tile([128, 16, 4, DH + 1], BF16, tag="v")
            nc.vector.memset(v_sb[:].rearrange("p a b c -> p (a b c)"), 1.0)

            # rope targets: [feat 128, S] per partition-tile, bf16
            qrot = [rope_pool.tile([128, S], BF16, tag="rope", name=f"qrot{i}")
                    for i in range(2)]
            krot = [rope_pool.tile([128, S], BF16, tag="rope", name=f"krot{i}")
                    for i in range(2)]

            def rope_big(qb, dst):
                # qb [128, S] bf16 bias-added projection; swap the 32-row
                # halves per head via DMA, then dst = qb*cos + swap*sinx.
                # Processed in two free-dim halves so the chain starts as
                # soon as the first half of qb exists.
                qbs = qbs_pool.tile([128, S], BF16, tag="qbs")
                for h in (0, 1):
                    hs = slice(h * 1024, (h + 1) * 1024)
                    for blk in (0, 1):
                        p0 = blk * 64
                        nc.sync.dma_start(out=qbs[p0:p0 + 32, hs],
                                            in_=qb[p0 + 32:p0 + 64, hs])
                        nc.sync.dma_start(out=qbs[p0 + 32:p0 + 64, hs],
                                            in_=qb[p0:p0 + 32, hs])
                    nc.vector.tensor_mul(out=dst[:, hs], in0=qb[:, hs],
                                         in1=cos2_sb[:, hs])
                    tmp = tmp_pool.tile([128, 1024], BF16, tag="tmp")
                    nc.vector.tensor_mul(out=tmp[:], in0=qbs[:, hs],
                                         in1=sinx_sb[:, hs])
                    nc.gpsimd.tensor_add(out=dst[:, hs], in0=dst[:, hs],
                                         in1=tmp[:])

            # ---- phase A: q/k projections + RoPE ----
            for proj in range(2):
                w_sb = (wq_sb, wk_sb)[proj]
                b_sb = (bq_sb, bk_sb)[proj]
                for pt in range(2):
                    qb = qb_pool.tile([128, S], BF16, tag="qb")
                    for st in range(NST):
                        ss = slice(st * 512, (st + 1) * 512)
                        ps = sc_pool.tile([128, 512], F32, tag="sc")
                        for kt in range(8):
                            nc.tensor.matmul(
                                ps[:], w_sb[:, kt, pt * 128:(pt + 1) * 128],
                                x_all[:, kt, ss],
                                start=(kt == 0), stop=(kt == 7))
                        nc.vector.tensor_scalar_add(
                            out=qb[:, ss], in0=ps[:],
                            scalar1=b_sb[:, pt:pt + 1])
                    if proj == 0:
                        rope_big(qb, qrot[pt])
                    else:
                        rope_big(qb, krot[pt])

            # ---- phase A': v projected position-major (x stationary) ----
            for sti in range(16):
                pv_ps = ps_pool.tile([128, FPC], F32, tag="ps")
                for kt in range(8):
                    nc.tensor.matmul(
                        pv_ps[:], x_all[:, kt, sti * 128:(sti + 1) * 128],
                        wv_sb[:, kt, :],
                        start=(kt == 0), stop=(kt == 7))
                nc.vector.tensor_add(
                    out=v_sb[:, sti, :, 0:DH],
                    in0=pv_ps[:].rearrange("p (h d) -> p h d", h=4),
                    in1=bv_bc[:].rearrange("p (h d) -> p h d", h=4))

            # ---- phase B: attention (qt-outer) ----
            woin = [woin_pool.tile([128, S], BF16, tag="woin",
                                   name=f"woin{i}") for i in range(2)]

            def scores(kt, qt, pair, qt0):
                ksl = slice(kt * 128, (kt + 1) * 128)
                off = max(0, kt * 128 - qt * 512)
                ps_s = sc_pool.tile([128, 1024], F32, tag="sc", name="ps_s")
                psv = ps_s[:].rearrange("p (h q) -> p h q", h=2)
                for h in (0, 1):
                    nc.tensor.matmul(
                        psv[:, h, off:512],
                        krot[pair][h * 64:(h + 1) * 64, ksl],
                        qrot[pair][h * 64:(h + 1) * 64, qt0 + off:qt0 + 512],
                        start=True, stop=True)
                pr = probs_pool.tile([128, 1024], BF16, tag="pr", name="pr")
                prv = pr[:].rearrange("p (h q) -> p h q", h=2)
                nc.scalar.activation(out=prv[:, :, off:512],
                                     in_=psv[:, :, off:512],
                                     func=EXP, scale=0.125)
                if kt * 128 >= qt * 512:
                    nc.gpsimd.affine_select(
                        out=prv[:, :, off:off + 128],
                        in_=prv[:, :, off:off + 128],
                        pattern=[[0, 2], [1, 128]],
                        compare_op=AOP.is_ge, fill=0.0,
                        base=0,
                        channel_multiplier=-1)
                return pr

            def pv(kt, pr, pv_a, pv_b, pair, nkt, qt):
                off = max(0, kt * 128 - qt * 512)
                prv = pr[:].rearrange("p (h q) -> p h q", h=2)
                nc.tensor.matmul(
                    pv_a[0:DH + 1, off:512], v_sb[:, kt, 2 * pair, :],
                    prv[:, 0, off:512],
                    start=(kt == 0), stop=(kt == nkt - 1))
                nc.tensor.matmul(
                    pv_b[0:DH + 1, off:512], v_sb[:, kt, 2 * pair + 1, :],
                    prv[:, 1, off:512],
                    start=(kt == 0), stop=(kt == nkt - 1))

            def wo_block(qt):
                qsl = slice(qt * 512, (qt + 1) * 512)
                for dm in range(8):
                    ps_o = ps_pool.tile([128, 512], F32, tag="ps",
                                        name="ps_o")
                    for pt in range(2):
                        nc.tensor.matmul(
                            ps_o[:], wo_sb[:, pt, dm * 128:(dm + 1) * 128],
                            woin[pt][:, qsl], start=(pt == 0), stop=(pt == 1))
                    ot = osb_pool.tile([128, QT], BF16, tag="ot")
                    nc.vector.tensor_scalar_add(
                        out=ot[:], in0=ps_o[:],
                        scalar1=bo_sb[:, dm:dm + 1])
                    nc.sync.dma_start(
                        out=out_e[dm * 128:(dm + 1) * 128, qsl], in_=ot[:])

            for qt in range(NQT):
                qt0 = qt * 512
                qsl = slice(qt * 512, (qt + 1) * 512)
                for pair in range(2):
                    pv_a = ps_pool.tile([DH + 1, 512], F32, tag="ps",
                                        name="pv_a")
                    pv_b = ps_pool.tile([DH + 1, 512], F32, tag="ps",
                                        name="pv_b")
                    nkt = 4 * qt + 4

                    # software pipeline: scores run one kt ahead of pv
                    pr_prev = scores(0, qt, pair, qt0)
                    for kt in range(1, nkt):
                        pr_k = scores(kt, qt, pair, qt0)
                        pv(kt - 1, pr_prev, pv_a, pv_b, pair, nkt, qt)
                        pr_prev = pr_k
                    pv(nkt - 1, pr_prev, pv_a, pv_b, pair, nkt, qt)

                    # denominator reciprocal + broadcast + normalize
                    rec = rec_pool.tile([1, 1024], F32, tag="rec")
                    nc.vector.reciprocal(
                        out=rec[0:1, 0:512], in_=pv_a[DH:DH + 1, :])
                    nc.vector.reciprocal(
                        out=rec[0:1, 512:1024], in_=pv_b[DH:DH + 1, :])
                    rb = rb_pool.tile([128, 1024], F32, tag="rb")
                    nc.gpsimd.partition_broadcast(rb[:], rec[0:1, :])
                    nc.vector.tensor_mul(out=woin[pair][0:64, qsl],
                                         in0=pv_a[0:DH, :],
                                         in1=rb[0:64, 0:512])
                    nc.vector.tensor_mul(out=woin[pair][64:128, qsl],
                                         in0=pv_b[0:DH, :],
                                         in1=rb[64:128, 512:1024])
                if qt > 0:
                    wo_block(qt - 1)
            wo_block(NQT - 1)

    nc.compile()
    return nc


def kernel(qkv, cos, sin, Wq, bq, Wk, bk, Wv, bv, Wo, bo):
    from concourse.bass_utils import run_bass_kernel_spmd

    qkv = np.asarray(qkv, dtype=np.float32)
    cos = np.asarray(cos, dtype=np.float32)
    sin = np.asarray(sin, dtype=np.float32)
    Wq, bq = np.asarray(Wq, np.float32), np.asarray(bq, np.float32)
    Wk, bk = np.asarray(Wk, np.float32), np.asarray(bk, np.float32)
    Wv, bv = np.asarray(Wv, np.float32), np.asarray(bv, np.float32)
    Wo, bo = np.asarray(Wo, np.float32), np.asarray(bo, np.float32)

    if "nc" not in _cache:
        _cache["nc"] = _build()
    nc = _cache["nc"]

    bf = ml_dtypes.bfloat16
    cos2 = np.ascontiguousarray(np.tile(cos.T, (2, 1)).astype(bf))  # [128, S]
    sinx = np.tile(sin.T, (2, 1))
    sinx[0:32] *= -1.0
    sinx[64:96] *= -1.0
    sinx = np.ascontiguousarray(sinx.astype(bf))

    bo4 = np.ascontiguousarray(bo * 0.25)
    in_maps = []
    for c in range(N_CORES):
        b, g = c // 4, c % 4
        hsl = slice(g * FPC, (g + 1) * FPC)
        in_maps.append({
            "qkvT": np.ascontiguousarray(qkv[b].T.astype(bf)),
            "wq": np.ascontiguousarray(Wq[hsl, :].T.astype(bf)),
            "wk": np.ascontiguousarray(Wk[hsl, :].T.astype(bf)),
            "wv": np.ascontiguousarray(Wv[hsl, :].T.astype(bf)),
            "wo": np.ascontiguousarray(Wo[:, hsl].T.astype(bf)),
            "bq": np.ascontiguousarray(bq[hsl]),
            "bk": np.ascontiguousarray(bk[hsl]),
            "bv": np.ascontiguousarray(bv[hsl][None, :]),
            "bo": bo4,
            "cos2": cos2,
            "sinx": sinx,
        })

    trace = bool(os.environ.get("KERNEL_TRACE"))
    res = run_bass_kernel_spmd(nc, in_maps, list(range(N_CORES)), trace=trace)
    last_run_info["exec_time_ns"] = res.exec_time_ns
    last_run_info["results"] = res

    out = np.empty((B, S, D), dtype=np.float32)
    for b in range(B):
        oT = (res.results[4 * b]["out"].astype(np.float32)
              + res.results[4 * b + 1]["out"].astype(np.float32)
              + res.results[4 * b + 2]["out"].astype(np.float32)
              + res.results[4 * b + 3]["out"].astype(np.float32))
        out[b] = oT.T
    return out


# revision 13
# speedup vs baseline: 1.4559x; 1.4559x over previous
"""Distributed causal RoPE attention for Trainium2 (8 NeuronCores).

Problem: nn_CausalRpeAttn — B=2, S=2048, D=1024, H=16, Dh=64, fp32.

Sharding (data + head parallel): core c handles batch c//4 and heads
4*(c%4) .. 4*(c%4)+3 (a 256-wide feature slice). Wq/Wk/Wv are split
column-wise (by output head group), Wo row-wise. Each core writes its
full [1024, 2048] (transposed) bf16 partial output projection (with
bo/4 pre-added); the host unshards by summing the 4 partials per batch
and transposing back. Attention itself is fully independent per
(batch, head), so the only cross-core combination is that final sum.

Key performance structure (v2):
 - Everything bf16 on the wires; fp32 only inside PSUM accumulation and
   the softmax denominator reciprocal.
 - q/k projections feature-major (moving dim 512); v is projected
   POSITION-major directly on the PE (x chunk as the stationary
   operand, Wv as moving) so no transposes are needed; bv is added
   during the PSUM->SBUF evacuation on the DVE against a pre-broadcast
   bias tile.
 - Scores are computed transposed sT[k, q], two heads of a 128-feature
   block run CONCURRENTLY on disjoint PE row groups (lhsT base
   partitions 0 / 64) into different PSUM banks. Causal-trimmed moving
   ranges everywhere.
 - exp on the Scalar engine only (it is the phase-B co-bottleneck);
   all copies/bias-adds live on DVE/GpSimd.
 - v carries an appended ones-row so PV yields the softmax denominator
   row; the pair's PV PSUM tile is evacuated to SBUF fp32 immediately
   (frees banks for the next pair), then reciprocal_approx_fast on the
   denominator row, gpsimd partition-broadcast, and two DVE muls
   produce the normalized bf16 Wo input. Wo runs one q-tile late so the
   PE never waits on that chain.
 - Input DMAs are split small (x in quarters per 128-partition chunk)
   and issued from four different engine queues so the first
   projection matmul starts ~6us in and transfers overlap compute.
"""

import os
import ml_dtypes
import numpy as np

B, S, D, H, DH = 2, 2048, 1024, 16, 64
N_CORES = 8
FPC = 256  # features per core (4 heads)
QT = 512
NQT = S // QT  # 4
NST = S // 512  # 4 s-tiles for projections

_cache = {}
last_run_info = {}


def _build():
    import concourse.bass as bass
    import concourse.mybir as mybir
    import concourse.tile as tile
    from concourse import bacc

    F32 = mybir.dt.float32
    BF16 = mybir.dt.bfloat16
    AOP = mybir.AluOpType
    EXP = mybir.ActivationFunctionType.Exp

    nc = bacc.Bacc("TRN2", target_bir_lowering=False, debug=False,
                   num_devices=N_CORES)

    qkvT_e = nc.dram_tensor("qkvT", [D, S], BF16, kind="ExternalInput").ap()
    wq_e = nc.dram_tensor("wq", [D, FPC], BF16, kind="ExternalInput").ap()
    wk_e = nc.dram_tensor("wk", [D, FPC], BF16, kind="ExternalInput").ap()
    wv_e = nc.dram_tensor("wv", [D, FPC], BF16, kind="ExternalInput").ap()
    wo_e = nc.dram_tensor("wo", [FPC, D], BF16, kind="ExternalInput").ap()
    bq_e = nc.dram_tensor("bq", [FPC], F32, kind="ExternalInput").ap()
    bk_e = nc.dram_tensor("bk", [FPC], F32, kind="ExternalInput").ap()
    bv_e = nc.dram_tensor("bv", [1, FPC], F32, kind="ExternalInput").ap()
    bo_e = nc.dram_tensor("bo", [D], F32, kind="ExternalInput").ap()
    cos2_e = nc.dram_tensor("cos2", [128, S], BF16, kind="ExternalInput").ap()
    sinx_e = nc.dram_tensor("sinx", [128, S], BF16, kind="ExternalInput").ap()
    out_e = nc.dram_tensor("out", [D, S], BF16, kind="ExternalOutput").ap()

    from contextlib import ExitStack
    with tile.TileContext(nc) as tc:
        with ExitStack() as ctx:
            ep = ctx.enter_context
            consts = ep(tc.tile_pool(name="consts", bufs=1))
            xin_pool = ep(tc.tile_pool(name="xin", bufs=1))
            rope_pool = ep(tc.tile_pool(name="rope", bufs=4))
            qb_pool = ep(tc.tile_pool(name="qb", bufs=2))
            qbs_pool = ep(tc.tile_pool(name="qbs", bufs=2))
            tmp_pool = ep(tc.tile_pool(name="tmp", bufs=2))
            vsb_pool = ep(tc.tile_pool(name="vsb", bufs=1))
            probs_pool = ep(tc.tile_pool(name="probs", bufs=4))
            woin_pool = ep(tc.tile_pool(name="woin", bufs=2))
            rec_pool = ep(tc.tile_pool(name="rec", bufs=2))
            rb_pool = ep(tc.tile_pool(name="rb", bufs=2))
            osb_pool = ep(tc.tile_pool(name="osb", bufs=3))
            # PSUM: sc 2x[128,1024] = 4 banks, ps 4x[128,512] = 4 banks.
            sc_pool = ep(tc.tile_pool(name="sc", bufs=2, space="PSUM"))
            ps_pool = ep(tc.tile_pool(name="ps", bufs=4, space="PSUM"))

            # ---- input DMAs, split fine and spread across issue queues ----
            wq_sb = consts.tile([128, 8, FPC], BF16, tag="wq")
            wk_sb = consts.tile([128, 8, FPC], BF16, tag="wk")
            wv_sb = consts.tile([128, 8, FPC], BF16, tag="wv")
            wq_r = wq_e.rearrange("(kt p) f -> p kt f", p=128)
            wk_r = wk_e.rearrange("(kt p) f -> p kt f", p=128)
            wv_r = wv_e.rearrange("(kt p) f -> p kt f", p=128)
            x_all = xin_pool.tile([128, 8, S], BF16, tag="x")

            # sync queue: wq interleaved with the first x quarter so the
            # q-projection can start streaming immediately.
            for kt in range(8):
                nc.sync.dma_start(out=wq_sb[:, kt, :], in_=wq_r[:, kt, :])
                nc.sync.dma_start(out=x_all[:, kt, 0:512],
                                  in_=qkvT_e[kt * 128:(kt + 1) * 128, 0:512])
            for kt in range(8):
                nc.sync.dma_start(out=x_all[:, kt, 512:1024],
                                  in_=qkvT_e[kt * 128:(kt + 1) * 128, 512:1024])

            # scalar queue: x quarter 2, then rope tables
            for kt in range(8):
                nc.scalar.dma_start(
                    out=x_all[:, kt, 1024:1536],
                    in_=qkvT_e[kt * 128:(kt + 1) * 128, 1024:1536])

            # scalar queue: rope tables (needed ~15us in)
            cos2_sb = consts.tile([128, S], BF16, tag="cos2")
            sinx_sb = consts.tile([128, S], BF16, tag="sinx")
            for h in (0, 1):
                hs = slice(h * 1024, (h + 1) * 1024)
                nc.scalar.dma_start(out=cos2_sb[:, hs], in_=cos2_e[:, hs])
                nc.scalar.dma_start(out=sinx_sb[:, hs], in_=sinx_e[:, hs])

            # gpsimd queue: x quarter 3, k/v/o weights + biases
            for kt in range(8):
                nc.gpsimd.dma_start(
                    out=x_all[:, kt, 1536:2048],
                    in_=qkvT_e[kt * 128:(kt + 1) * 128, 1536:2048])
            b_sbs = []
            for name, be in (("bq", bq_e), ("bk", bk_e)):
                t = consts.tile([128, 2], F32, tag=name, name=name)
                nc.gpsimd.dma_start(out=t[:],
                                    in_=be.rearrange("(t p) -> p t", p=128))
                b_sbs.append(t)
            bq_sb, bk_sb = b_sbs
            bv_row = consts.tile([1, FPC], F32, tag="bvr")
            nc.gpsimd.dma_start(out=bv_row[:], in_=bv_e)
            bo_sb = consts.tile([128, 8], F32, tag="bo")
            nc.gpsimd.dma_start(out=bo_sb[:],
                                in_=bo_e.rearrange("(t p) -> p t", p=128))
            for kt in range(0, 8, 2):
                nc.gpsimd.dma_start(out=wk_sb[:, kt:kt + 2, :],
                                    in_=wk_r[:, kt:kt + 2, :])
            for kt in range(0, 8, 2):
                nc.gpsimd.dma_start(out=wv_sb[:, kt:kt + 2, :],
                                    in_=wv_r[:, kt:kt + 2, :])
            wo_sb = consts.tile([128, 2, D], BF16, tag="wo")
            wo_r = wo_e.rearrange("(pt p) f -> p pt f", p=128)
            for pt in range(2):
                nc.gpsimd.dma_start(out=wo_sb[:, pt, :], in_=wo_r[:, pt, :])

            bv_bc = consts.tile([128, FPC], F32, tag="bvb")
            nc.gpsimd.partition_broadcast(bv_bc[:], bv_row[0:1, :])

            # v with ones row: [pos 128, 16 s-tiles, 4 heads, 64+1]
            v_sb = vsb_pool.tile([128, 16, 4, DH + 1], BF16, tag="v")
            nc.vector.memset(v_sb[:].rearrange("p a b c -> p (a b c)"), 1.0)

            # rope targets: [feat 128, S] per partition-tile, bf16
            qrot = [rope_pool.tile([128, S], BF16, tag="rope", name=f"qrot{i}")
                    for i in range(2)]
            krot = [rope_pool.tile([128, S], BF16, tag="rope", name=f"krot{i}")
                    for i in range(2)]

            def rope_big(qb, dst):
                # qb [128, S] bf16 bias-added projection; swap the 32-row
                # halves per head via DMA, then dst = qb*cos + swap*sinx.
                # Processed in two free-dim halves so the chain starts as
                # soon as the first half of qb exists.
                qbs = qbs_pool.tile([128, S], BF16, tag="qbs")
                for h in (0, 1):
                    hs = slice(h * 1024, (h + 1) * 1024)
                    for blk in (0, 1):
                        p0 = blk * 64
                        nc.scalar.dma_start(out=qbs[p0:p0 + 32, hs],
                                            in_=qb[p0 + 32:p0 + 64, hs])
                        nc.scalar.dma_start(out=qbs[p0 + 32:p0 + 64, hs],
                                            in_=qb[p0:p0 + 32, hs])
                    nc.vector.tensor_mul(out=dst[:, hs], in0=qb[:, hs],
                                         in1=cos2_sb[:, hs])
                    tmp = tmp_pool.tile([128, 1024], BF16, tag="tmp")
                    nc.vector.tensor_mul(out=tmp[:], in0=qbs[:, hs],
                                         in1=sinx_sb[:, hs])
                    nc.gpsimd.tensor_add(out=dst[:, hs], in0=dst[:, hs],
                                         in1=tmp[:])

            # ---- phase A: q/k projections + RoPE ----
            for proj in range(2):
                w_sb = (wq_sb, wk_sb)[proj]
                b_sb = (bq_sb, bk_sb)[proj]
                for pt in range(2):
                    qb = qb_pool.tile([128, S], BF16, tag="qb")
                    for st in range(NST):
                        ss = slice(st * 512, (st + 1) * 512)
                        ps = sc_pool.tile([128, 512], F32, tag="sc")
                        for kt in range(8):
                            nc.tensor.matmul(
                                ps[:], w_sb[:, kt, pt * 128:(pt + 1) * 128],
                                x_all[:, kt, ss],
                                start=(kt == 0), stop=(kt == 7))
                        nc.vector.tensor_scalar_add(
                            out=qb[:, ss], in0=ps[:],
                            scalar1=b_sb[:, pt:pt + 1])
                    if proj == 0:
                        rope_big(qb, qrot[pt])
                    else:
                        rope_big(qb, krot[pt])

            # ---- phase A': v projected position-major (x stationary) ----
            for sti in range(16):
                pv_ps = ps_pool.tile([128, FPC], F32, tag="ps")
                for kt in range(8):
                    nc.tensor.matmul(
                        pv_ps[:], x_all[:, kt, sti * 128:(sti + 1) * 128],
                        wv_sb[:, kt, :],
                        start=(kt == 0), stop=(kt == 7))
                nc.vector.tensor_add(
                    out=v_sb[:, sti, :, 0:DH],
                    in0=pv_ps[:].rearrange("p (h d) -> p h d", h=4),
                    in1=bv_bc[:].rearrange("p (h d) -> p h d", h=4))

            # ---- phase B: attention (qt-outer) ----
            woin = [woin_pool.tile([128, S], BF16, tag="woin",
                                   name=f"woin{i}") for i in range(2)]

            def scores(kt, qt, pair, qt0):
                ksl = slice(kt * 128, (kt + 1) * 128)
                off = max(0, kt * 128 - qt * 512)
                ps_s = sc_pool.tile([128, 1024], F32, tag="sc", name="ps_s")
                psv = ps_s[:].rearrange("p (h q) -> p h q", h=2)
                for h in (0, 1):
                    nc.tensor.matmul(
                        psv[:, h, off:512],
                        krot[pair][h * 64:(h + 1) * 64, ksl],
                        qrot[pair][h * 64:(h + 1) * 64, qt0 + off:qt0 + 512],
                        start=True, stop=True)
                pr = probs_pool.tile([128, 1024], BF16, tag="pr", name="pr")
                prv = pr[:].rearrange("p (h q) -> p h q", h=2)
                nc.scalar.activation(out=prv[:, :, off:512],
                                     in_=psv[:, :, off:512],
                                     func=EXP, scale=0.125)
                if kt * 128 >= qt * 512:
                    nc.gpsimd.affine_select(
                        out=prv[:, :, off:off + 128],
                        in_=prv[:, :, off:off + 128],
                        pattern=[[0, 2], [1, 128]],
                        compare_op=AOP.is_ge, fill=0.0,
                        base=0,
                        channel_multiplier=-1)
                return pr

            def pv(kt, pr, pv_a, pv_b, pair, nkt, qt):
                off = max(0, kt * 128 - qt * 512)
                prv = pr[:].rearrange("p (h q) -> p h q", h=2)
                nc.tensor.matmul(
                    pv_a[0:DH + 1, off:512], v_sb[:, kt, 2 * pair, :],
                    prv[:, 0, off:512],
                    start=(kt == 0), stop=(kt == nkt - 1))
                nc.tensor.matmul(
                    pv_b[0:DH + 1, off:512], v_sb[:, kt, 2 * pair + 1, :],
                    prv[:, 1, off:512],
                    start=(kt == 0), stop=(kt == nkt - 1))

            def wo_block(qt):
                qsl = slice(qt * 512, (qt + 1) * 512)
                for dm in range(8):
                    ps_o = ps_pool.tile([128, 512], F32, tag="ps",
                                        name="ps_o")
                    for pt in range(2):
                        nc.tensor.matmul(
                            ps_o[:], wo_sb[:, pt, dm * 128:(dm + 1) * 128],
                            woin[pt][:, qsl], start=(pt == 0), stop=(pt == 1))
                    ot = osb_pool.tile([128, QT], BF16, tag="ot")
                    nc.vector.tensor_scalar_add(
                        out=ot[:], in0=ps_o[:],
                        scalar1=bo_sb[:, dm:dm + 1])
                    nc.sync.dma_start(
                        out=out_e[dm * 128:(dm + 1) * 128, qsl], in_=ot[:])

            for qt in range(NQT):
                qt0 = qt * 512
                qsl = slice(qt * 512, (qt + 1) * 512)
                for pair in range(2):
                    pv_a = ps_pool.tile([DH + 1, 512], F32, tag="ps",
                                        name="pv_a")
                    pv_b = ps_pool.tile([DH + 1, 512], F32, tag="ps",
                                        name="pv_b")
                    nkt = 4 * qt + 4

                    # software pipeline: scores run one kt ahead of pv
                    pr_prev = scores(0, qt, pair, qt0)
                    for kt in range(1, nkt):
                        pr_k = scores(kt, qt, pair, qt0)
                        pv(kt - 1, pr_prev, pv_a, pv_b, pair, nkt, qt)
                        pr_prev = pr_k
                    pv(nkt - 1, pr_prev, pv_a, pv_b, pair, nkt, qt)

                    # denominator reciprocal + broadcast + normalize
                    den = rec_pool.tile([1, 1024], F32, tag="den",
                                        name="den")
                    nc.vector.tensor_copy(out=den[0:1, 0:512],
                                          in_=pv_a[DH:DH + 1, :])
                    nc.vector.tensor_copy(out=den[0:1, 512:1024],
                                          in_=pv_b[DH:DH + 1, :])
                    rec = rec_pool.tile([1, 1024], F32, tag="rec")
                    nc.vector.reciprocal_approx_fast(
                        out=rec[0:1, :], in_=den[0:1, :])
                    rb = rb_pool.tile([128, 1024], F32, tag="rb")
                    nc.gpsimd.partition_broadcast(rb[:], rec[0:1, :])
                    nc.vector.tensor_mul(out=woin[pair][0:64, qsl],
                                         in0=pv_a[0:DH, :],
                                         in1=rb[0:64, 0:512])
                    nc.vector.tensor_mul(out=woin[pair][64:128, qsl],
                                         in0=pv_b[0:DH, :],
                                         in1=rb[64:128, 512:1024])
                if qt > 0:
                    wo_block(qt - 1)
            wo_block(NQT - 1)

    nc.compile()
    return nc


def kernel(qkv, cos, sin, Wq, bq, Wk, bk, Wv, bv, Wo, bo):
    from concourse.bass_utils import run_bass_kernel_spmd

    qkv = np.asarray(qkv, dtype=np.float32)
    cos = np.asarray(cos, dtype=np.float32)
    sin = np.asarray(sin, dtype=np.float32)
    Wq, bq = np.asarray(Wq, np.float32), np.asarray(bq, np.float32)
    Wk, bk = np.asarray(Wk, np.float32), np.asarray(bk, np.float32)
    Wv, bv = np.asarray(Wv, np.float32), np.asarray(bv, np.float32)
    Wo, bo = np.asarray(Wo, np.float32), np.asarray(bo, np.float32)

    if "nc" not in _cache:
        _cache["nc"] = _build()
    nc = _cache["nc"]

    bf = ml_dtypes.bfloat16
    cos2 = np.ascontiguousarray(np.tile(cos.T, (2, 1)).astype(bf))  # [128, S]
    sinx = np.tile(sin.T, (2, 1))
    sinx[0:32] *= -1.0
    sinx[64:96] *= -1.0
    sinx = np.ascontiguousarray(sinx.astype(bf))

    bo4 = np.ascontiguousarray(bo * 0.25)
    in_maps = []
    for c in range(N_CORES):
        b, g = c // 4, c % 4
        hsl = slice(g * FPC, (g + 1) * FPC)
        in_maps.append({
            "qkvT": np.ascontiguousarray(qkv[b].T.astype(bf)),
            "wq": np.ascontiguousarray(Wq[hsl, :].T.astype(bf)),
            "wk": np.ascontiguousarray(Wk[hsl, :].T.astype(bf)),
            "wv": np.ascontiguousarray(Wv[hsl, :].T.astype(bf)),
            "wo": np.ascontiguousarray(Wo[:, hsl].T.astype(bf)),
            "bq": np.ascontiguousarray(bq[hsl]),
            "bk": np.ascontiguousarray(bk[hsl]),
            "bv": np.ascontiguousarray(bv[hsl][None, :]),
            "bo": bo4,
            "cos2": cos2,
            "sinx": sinx,
        })

    trace = bool(os.environ.get("KERNEL_TRACE"))
    res = run_bass_kernel_spmd(nc, in_maps, list(range(N_CORES)), trace=trace)
    last_run_info["exec_time_ns"] = res.exec_time_ns
    last_run_info["results"] = res

    out = np.empty((B, S, D), dtype=np.float32)
    for b in range(B):
        oT = (res.results[4 * b]["out"].astype(np.float32)
              + res.results[4 * b + 1]["out"].astype(np.float32)
              + res.results[4 * b + 2]["out"].astype(np.float32)
              + res.results[4 * b + 3]["out"].astype(np.float32))
        out[b] = oT.T
    return out
